# revision 11
# baseline (speedup 1.0000x reference)
"""EvolveGCN-reg Trainium2 kernel (8 NeuronCores, timestep-parallel).

Math: the reference computes, per timestep t (scan carrying a GRU-evolved
16x16 weight W):
    y_t   = X_t @ p / ||p||;  (yk, idx) = top16(y_t);  Xs = (X_t[idx] * yk).T
    W_t   = GRU(W_{t-1}, Xs)          (16x16 matmuls, tiny)
    AH    = segment_sum(val * X_t[col], row, N)       (3.2M-edge sparse op)
    out_t = (AH @ W_t) @ lin_w + b
Key identity used here:  out_t = A_t @ (X_t @ (W_t @ lin_w)) + b, so the
feature dimension collapses and the sparse phase is a scalar gather /
segment-sum:  out_t[n] = b + sum_{e: row[e]=n} val[e] * s_t[col[e]],
with s_t = X_t @ u_t and u_t = W_t @ lin_w.

Sharding: core t owns timestep t (uniform load, no collectives). Host does
index-space layout only: edges are grouped by destination, destinations
degree-sorted and assigned round-robin to the 128 SBUF partitions so each
"rank" of 128 nodes shares a common padded segment length. The segment-sum
then becomes strided tensor_reduce ops at line rate, split across the DVE
and GpSimd engines. The one index permutation (s gathered edge-wise)
happens during host-side re-staging between device launches; every
floating-point operation of the model runs on the NeuronCores.

Device launches (all math on device):
  L1: yraw_t = X_t @ p  (fp32 - top-16 selection order is precision
      critical) -> host extracts top-16 *indices*
  L2: ||p|| normalize, Xs_tau, full GRU chain (Z|R gates stacked), u_t
      select, s_t = X_t @ u_t from a bf16 copy of X; s written as bf16
  L3: w = val * s[col] (bf16 streams, gathered layout), per-rank
      segmented reduce in fp32, + b

Throughput structure: big streams alternate between the two HWDGE DMA
queues (sync / scalar) so per-DMA completion latency is hidden, and the
elementwise work is split DVE / GpSimd so compute stays under the DMA
line rate.
"""

import numpy as np
from contextlib import ExitStack

import ml_dtypes

import concourse.bass as bass
import concourse.bacc as bacc
import concourse.tile as tile
from concourse import mybir
from concourse.bass_utils import run_bass_kernel_spmd

dt = mybir.dt
bf16 = ml_dtypes.bfloat16

T, N, E, F0, F1 = 8, 100000, 3200000, 16, 16
NCORES = 8
P = 128
RANKS = (N + P - 1) // P  # 782
N_PAD = P * RANKS  # 100096
CORE_IDS = list(range(NCORES))

_cache = {}


def _axon_reset():
    try:
        import ctypes

        lib = ctypes.CDLL("/opt/axon/libaxon_pjrt.so")
        lib.axon_reset.restype = ctypes.c_int64
        lib.axon_reset()
    except Exception:
        pass


def _run(nc, in_maps):
    try:
        return run_bass_kernel_spmd(nc, in_maps, core_ids=CORE_IDS)
    except Exception:
        _axon_reset()
        return run_bass_kernel_spmd(nc, in_maps, core_ids=CORE_IDS)


def _emit_matvec(nc, io, acc_pool, xt_ap, u, out_sb, in_dt, tag=""):
    """out_sb[p, r] = sum_f X_T[p, f, r] * u[f].

    X streamed in 4 feature-chunks alternating the two HWDGE queues.
    `u` is either a host float vector (L1: scalars baked as immediates,
    so GpSimd can run fused stt chains too) or a [P, F0] SBUF AP (L2:
    DVE uses the scalar-ptr stt; GpSimd gets broadcast-mult + add since
    Pool rejects TensorScalarPtr).

    Chain layout (feature f of chunk c = 4c + k):
      imm:  DVE chains a(k=0), b(k=1); GpSimd chains g(k=2), h(k=3)
      ptr:  DVE chains a, b, g (k=0,1,2); GpSimd chain h (k=3)
    """
    imm = not isinstance(u, bass.AP)
    xt = io.tile([P, F0, RANKS], in_dt, tag=f"xt{tag}", name=f"xt{tag}")
    NCH = 4
    FC = F0 // NCH
    for c in range(NCH):
        eng = nc.sync if c % 2 == 0 else nc.scalar
        eng.dma_start(
            xt[:, c * FC : (c + 1) * FC, :],
            xt_ap[:, c * FC * RANKS : (c + 1) * FC * RANKS].rearrange(
                "p (f r) -> p f r", r=RANKS),
        )

    def tl(k):
        return acc_pool.tile([P, RANKS], dt.float32, tag=f"acc_{k}{tag}",
                             name=f"acc_{k}{tag}")

    # DVE: fused mult-add chains a/b/c (10 feats). GpSimd has no fused
    # scalar op: mult (imm tensor_scalar / broadcast tensor_tensor) into a
    # tmp, then add - chains g/h (6 feats). Feature lists are in chunk
    # arrival order so both engines advance as each DMA chunk lands.
    DVE_CH = [[0, 4, 8, 12], [1, 5, 9, 13], [2, 10]]
    GP_CH = [[3, 7, 14], [6, 11, 15]]
    dve_accs = [tl(k) for k in ("a", "b", "c")]
    gp_accs = [tl(k) for k in ("g", "h")]
    gp_tmps = [tl(k) for k in ("tg", "th")]

    ops = []  # (feat, emit_fn) merged in feature order
    for ci, chain in enumerate(DVE_CH):
        for j, f in enumerate(chain):
            def emit(f=f, ci=ci, first=j == 0):
                s = float(u[f]) if imm else u[:, f : f + 1]
                acc = dve_accs[ci]
                if first:
                    nc.vector.tensor_scalar_mul(acc[:], xt[:, f, :], s)
                else:
                    nc.vector.scalar_tensor_tensor(
                        out=acc[:], in0=xt[:, f, :], scalar=s, in1=acc[:],
                        op0=mybir.AluOpType.mult, op1=mybir.AluOpType.add,
                    )
            ops.append((f, emit))
    for ci, chain in enumerate(GP_CH):
        for j, f in enumerate(chain):
            def emit(f=f, ci=ci, first=j == 0):
                acc, tmp = gp_accs[ci], gp_tmps[ci]
                dst = acc if first else tmp
                if imm:
                    nc.gpsimd.tensor_scalar_mul(dst[:], xt[:, f, :], float(u[f]))
                else:
                    nc.gpsimd.tensor_tensor(
                        out=dst[:], in0=xt[:, f, :],
                        in1=u[:, f : f + 1].broadcast_to([P, RANKS]),
                        op=mybir.AluOpType.mult)
                if not first:
                    nc.gpsimd.tensor_tensor(out=acc[:], in0=acc[:], in1=tmp[:],
                                            op=mybir.AluOpType.add)
            ops.append((f, emit))
    for _, emit in sorted(ops, key=lambda x: x[0]):
        emit()
    nc.vector.tensor_tensor(out=dve_accs[0][:], in0=dve_accs[0][:],
                            in1=dve_accs[1][:], op=mybir.AluOpType.add)
    nc.gpsimd.tensor_tensor(out=gp_accs[0][:], in0=gp_accs[0][:],
                            in1=gp_accs[1][:], op=mybir.AluOpType.add)
    nc.vector.tensor_tensor(out=dve_accs[0][:], in0=dve_accs[0][:],
                            in1=dve_accs[2][:], op=mybir.AluOpType.add)
    nc.vector.tensor_tensor(out=out_sb[:], in0=dve_accs[0][:],
                            in1=gp_accs[0][:], op=mybir.AluOpType.add)


# ---------------------------------------------------------------- launch 1
def _build_p1(p_vec):
    nc = bacc.Bacc("TRN2", target_bir_lowering=False, debug=False)
    xt_ap = nc.dram_tensor("XT", [P, F0 * RANKS], dt.float32, kind="ExternalInput").ap()
    y_ap = nc.dram_tensor("yraw", [P, RANKS], dt.float32, kind="ExternalOutput").ap()

    with tile.TileContext(nc) as tc, ExitStack() as ctx:
        io = ctx.enter_context(tc.tile_pool(name="io", bufs=1))
        yp = ctx.enter_context(tc.tile_pool(name="y", bufs=1))
        y_t = yp.tile([P, RANKS], dt.float32)
        _emit_matvec(nc, io, yp, xt_ap, p_vec, y_t, dt.float32)
        nc.sync.dma_start(y_ap[:], y_t[:])
    nc.compile()
    return nc


# ---------------------------------------------------------------- launch 2
# packed small-input column layout (one [32, .] DMA): see kernel()
_COLS = {}
_off = 0
for _n, _w in [("X16", 128), ("yraw16", 8), ("WZT", 16), ("UZT", 16), ("BZT", 16),
               ("WRT", 16), ("URT", 16), ("BRT", 16), ("WHT", 16), ("UHT", 16),
               ("BHT", 16), ("Winit", 16), ("I16", 16), ("linw_rep", 16),
               ("sel", 8), ("prep16", 16)]:
    _COLS[_n] = (_off, _off + _w)
    _off += _w
SMALLS_W = _off


def _build_p2():
    nc = bacc.Bacc("TRN2", target_bir_lowering=False, debug=False)
    xt_ap = nc.dram_tensor("XT", [P, F0 * RANKS], dt.bfloat16, kind="ExternalInput").ap()
    sm_ap = nc.dram_tensor("smalls", [16, SMALLS_W], dt.float32, kind="ExternalInput").ap()
    s_ap = nc.dram_tensor("s", [P, RANKS], dt.bfloat16, kind="ExternalOutput").ap()

    with tile.TileContext(nc) as tc, ExitStack() as ctx:
        small = ctx.enter_context(tc.tile_pool(name="small", bufs=1))
        gru = ctx.enter_context(tc.tile_pool(name="gru", bufs=2))
        ps = ctx.enter_context(tc.tile_pool(name="ps", bufs=2, space="PSUM"))
        psxs = ctx.enter_context(tc.tile_pool(name="psxs", bufs=2, space="PSUM"))
        io = ctx.enter_context(tc.tile_pool(name="io", bufs=1))
        sp = ctx.enter_context(tc.tile_pool(name="s", bufs=1))

        sm = small.tile([16, SMALLS_W], dt.float32)
        nc.scalar.dma_start(sm[:], sm_ap[:])

        # prefetch ACT function tables while the DMA is in flight
        warm = small.tile([1, 2], dt.float32)
        nc.vector.memset(warm[:], 0.0)
        nc.scalar.activation(warm[:, 0:1], warm[:, 0:1],
                             mybir.ActivationFunctionType.Sigmoid)
        nc.scalar.activation(warm[:, 1:2], warm[:, 1:2],
                             mybir.ActivationFunctionType.Tanh)

        def gi(name):
            a, b = _COLS[name]
            return sm[:, a:b]

        # invp = 1/||p|| on partition 0, replicated to 16 partitions via PE
        psq = small.tile([1, F0], dt.float32)
        nc.scalar.square(psq[:], gi("prep16")[0:1, :])
        pss = small.tile([1, 1], dt.float32)
        nc.vector.tensor_reduce(out=pss[:], in_=psq[:], axis=mybir.AxisListType.X,
                                op=mybir.AluOpType.add)
        pnorm = small.tile([1, 1], dt.float32)
        nc.scalar.sqrt(pnorm[:], pss[:])
        invp = small.tile([1, 1], dt.float32)
        nc.vector.reciprocal(invp[:], pnorm[:])
        ones1x16 = small.tile([1, 16], dt.float32)
        nc.vector.memset(ones1x16[:], 1.0)
        invp16_ps = ps.tile([16, 1], dt.float32, tag="misc", name="invp16_ps")
        nc.tensor.matmul(invp16_ps[:], ones1x16[:], invp[:], start=True, stop=True)
        invp16 = small.tile([16, 1], dt.float32)
        nc.scalar.copy(invp16[:], invp16_ps[:])

        # Xs_tau = (X16_tau^T @ diag(yraw_tau)) * invp   (scale fused in copy)
        Xs = []
        for tau in range(T):
            dg = gru.tile([16, 16], dt.float32, tag="diag", name=f"dg{tau}")
            nc.vector.tensor_scalar_mul(dg[:], gi("I16"),
                                        gi("yraw16")[:, tau : tau + 1])
            xs_ps = psxs.tile([16, 16], dt.float32, tag="xs", name=f"xsps{tau}")
            nc.tensor.matmul(xs_ps[:], gi("X16")[:, tau * F0 : (tau + 1) * F0],
                             dg[:], start=True, stop=True)
            xs = gru.tile([16, 16], dt.float32, tag="xs_sb", name=f"xs{tau}")
            nc.vector.tensor_scalar_mul(xs[:], xs_ps[:], invp16[:])
            Xs.append(xs)

        # GRU chain; bias folded into the PE accumulation group
        u_cols = small.tile([16, T], dt.float32)
        W = gi("Winit")
        for tau in range(T):
            def gate(wt, ut, bt, rhs2, func, tag):
                acc = ps.tile([16, 16], dt.float32, tag="mm", name=f"mm{tag}{tau}")
                nc.tensor.matmul(acc[:], gi(wt), Xs[tau][:], start=True, stop=False)
                nc.tensor.matmul(acc[:], gi(bt), gi("I16"), start=False, stop=False)
                nc.tensor.matmul(acc[:], gi(ut), rhs2[:], start=False, stop=True)
                g = gru.tile([16, 16], dt.float32, tag=f"g{tag}", name=f"g{tag}{tau}")
                nc.scalar.activation(g[:], acc[:], func)
                return g

            Zg = gate("WZT", "UZT", "BZT", W, mybir.ActivationFunctionType.Sigmoid, "z")
            Rg = gate("WRT", "URT", "BRT", W, mybir.ActivationFunctionType.Sigmoid, "r")
            RW = gru.tile([16, 16], dt.float32, tag="rw", name=f"rw{tau}")
            nc.vector.tensor_tensor(out=RW[:], in0=Rg[:], in1=W[:],
                                    op=mybir.AluOpType.mult)
            Ht = gate("WHT", "UHT", "BHT", RW, mybir.ActivationFunctionType.Tanh, "h")

            HmW = gru.tile([16, 16], dt.float32, tag="hmw", name=f"hmw{tau}")
            nc.vector.tensor_tensor(out=HmW[:], in0=Ht[:], in1=W[:],
                                    op=mybir.AluOpType.subtract)
            ZH = gru.tile([16, 16], dt.float32, tag="zh", name=f"zh{tau}")
            nc.vector.tensor_tensor(out=ZH[:], in0=Zg[:], in1=HmW[:],
                                    op=mybir.AluOpType.mult)
            Wn = gru.tile([16, 16], dt.float32, tag=f"w{tau}", name=f"w{tau}")
            nc.vector.tensor_tensor(out=Wn[:], in0=W[:], in1=ZH[:],
                                    op=mybir.AluOpType.add)
            W = Wn

            um = gru.tile([16, 16], dt.float32, tag="um", name=f"um{tau}")
            nc.vector.tensor_tensor(out=um[:], in0=W[:], in1=gi("linw_rep"),
                                    op=mybir.AluOpType.mult)
            nc.vector.tensor_reduce(out=u_cols[:, tau : tau + 1], in_=um[:],
                                    axis=mybir.AxisListType.X, op=mybir.AluOpType.add)

        # select this core's u via one-hot input mask; broadcast to 128 parts
        usm = small.tile([16, T], dt.float32)
        nc.vector.tensor_tensor(out=usm[:], in0=u_cols[:], in1=gi("sel"),
                                op=mybir.AluOpType.mult)
        u_sel = small.tile([16, 1], dt.float32)
        nc.vector.tensor_reduce(out=u_sel[:], in_=usm[:], axis=mybir.AxisListType.X,
                                op=mybir.AluOpType.add)
        diag_u = small.tile([16, 16], dt.float32)
        nc.vector.tensor_scalar_mul(diag_u[:], gi("I16"), u_sel[:])
        ones16x128 = small.tile([16, P], dt.float32)
        nc.vector.memset(ones16x128[:], 1.0)
        ub_ps = ps.tile([P, 16], dt.float32, tag="misc", name="ub_ps")
        nc.tensor.matmul(ub_ps[:], ones16x128[:], diag_u[:], start=True, stop=True)
        ub = small.tile([P, 16], dt.float32)
        nc.scalar.copy(ub[:], ub_ps[:])

        s_t = sp.tile([P, RANKS], dt.bfloat16)
        _emit_matvec(nc, io, sp, xt_ap, ub, s_t, dt.bfloat16)
        nc.sync.dma_start(s_ap[:], s_t[:])
    nc.compile()
    return nc


# ---------------------------------------------------------------- launch 3
def _build_p3(Ls, chunks, f_pad):
    nc = bacc.Bacc("TRN2", target_bir_lowering=False, debug=False)
    in_dt = dt.bfloat16
    tot = sum(sum(L * cnt for (L, cnt, _) in runs) for _, runs in chunks) * P
    sg_ap = nc.dram_tensor("sg", [tot], in_dt, kind="ExternalInput").ap()
    val_ap = nc.dram_tensor("val", [tot], in_dt, kind="ExternalInput").ap()
    b_ap = nc.dram_tensor("linb", [P, 1], dt.float32, kind="ExternalInput").ap()
    y_ap = nc.dram_tensor("y", [P, RANKS], dt.float32, kind="ExternalOutput").ap()

    with tile.TileContext(nc) as tc, ExitStack() as ctx:
        io = ctx.enter_context(tc.tile_pool(name="io", bufs=3))
        yp = ctx.enter_context(tc.tile_pool(name="y", bufs=1))
        b_t = yp.tile([P, 1], dt.float32)
        nc.scalar.dma_start(b_t[:], b_ap[:])
        y_t = yp.tile([P, RANKS], dt.float32)
        for ci, (col0, runs) in enumerate(chunks):
            ncols = sum(L * cnt for (L, cnt, _) in runs)
            sg_t = io.tile([P, ncols], in_dt, tag="sg", name="sg_t")
            nc.sync.dma_start(
                sg_t[:], sg_ap[col0 * P : (col0 + ncols) * P].rearrange(
                    "(p j) -> p j", j=ncols))
            val_t = io.tile([P, ncols], in_dt, tag="val", name="val_t")
            nc.scalar.dma_start(
                val_t[:], val_ap[col0 * P : (col0 + ncols) * P].rearrange(
                    "(p j) -> p j", j=ncols))
            w_t = io.tile([P, ncols], in_dt, tag="w", name="w_t")
            # GpSimd owns the elementwise product (Pool rejects free-axis
            # reduce); DVE owns the segmented reduces. Product split in two
            # halves so the first reduces start at half-chunk latency.
            half = runs[: (len(runs) + 1) // 2]
            hcols = sum(L * cnt for (L, cnt, _) in half)
            nc.gpsimd.tensor_tensor(
                out=w_t[:, :hcols], in0=sg_t[:, :hcols],
                in1=val_t[:, :hcols], op=mybir.AluOpType.mult)
            if hcols < ncols:
                nc.gpsimd.tensor_tensor(
                    out=w_t[:, hcols:], in0=sg_t[:, hcols:],
                    in1=val_t[:, hcols:], op=mybir.AluOpType.mult)
            c = 0
            for L, cnt, rank0 in runs:
                seg = w_t[:, c : c + cnt * L].rearrange("p (r l) -> p r l", l=L)
                nc.vector.tensor_reduce(
                    out=y_t[:, rank0 : rank0 + cnt], in_=seg,
                    axis=mybir.AxisListType.X, op=mybir.AluOpType.add,
                )
                c += cnt * L
        yb = yp.tile([P, RANKS], dt.float32)
        nc.vector.tensor_scalar_add(yb[:], y_t[:], b_t[:])
        nc.sync.dma_start(y_ap[:], yb[:])
    nc.compile()
    return nc


# ------------------------------------------------------------ host layout
def _edge_layout(edge_row, edge_col, edge_val):
    """Degree-sorted, rank-equalized destination layout shared across T."""
    degs = np.zeros((T, N_PAD), np.int64)
    orders = np.zeros((T, N_PAD), np.int64)
    for t in range(T):
        deg = np.bincount(edge_row[t].astype(np.int64), minlength=N_PAD)
        degs[t] = deg
        orders[t] = np.argsort(-deg, kind="stable")
    rank_max = np.zeros((T, RANKS), np.int64)
    for t in range(T):
        rank_max[t] = degs[t][orders[t]].reshape(RANKS, P).max(1)
    Ls = rank_max.max(0)
    Ls = np.maximum.accumulate(Ls[::-1])[::-1]  # enforce non-increasing
    Ls = np.maximum(Ls, 1)
    offs = np.zeros(RANKS + 1, np.int64)
    offs[1:] = np.cumsum(Ls)
    f_pad = int(-(-offs[-1] // 8) * 8)

    col_layout = np.zeros((T, P, f_pad), np.int32)
    val_layout = np.zeros((T, P, f_pad), np.float32)
    for t in range(T):
        row = edge_row[t].astype(np.int64)
        order = orders[t]
        slot_of_node = np.empty(N_PAD, np.int64)
        slot_of_node[order] = np.arange(N_PAD)
        ord_e = np.argsort(row, kind="stable")
        rows_s = row[ord_e]
        deg = degs[t]
        node_start = np.zeros(N_PAD, np.int64)
        node_start[1:] = np.cumsum(deg)[:-1]
        k = np.arange(E, dtype=np.int64) - node_start[rows_s]
        s = slot_of_node[rows_s]
        p_idx = s % P
        r_idx = s // P
        pos = offs[r_idx] + k
        col_layout[t, p_idx, pos] = edge_col[t][ord_e]
        val_layout[t, p_idx, pos] = edge_val[t][ord_e]

    # chunk schedule shared across cores (columns are bf16 on device)
    FC = 6400
    chunks = []
    cur, cur_cols, col0, r = [], 0, 0, 0
    while r < RANKS:
        L = int(Ls[r])
        cnt = 0
        while r + cnt < RANKS and Ls[r + cnt] == L and cur_cols + (cnt + 1) * L <= FC:
            cnt += 1
        if cnt == 0:
            chunks.append((col0, cur))
            col0 += cur_cols
            cur, cur_cols = [], 0
            continue
        cur.append((L, cnt, r))
        cur_cols += cnt * L
        r += cnt
    if cur:
        chunks.append((col0, cur))
    return Ls, offs, f_pad, col_layout, val_layout, orders, chunks


# ------------------------------------------------------------------ kernel
def kernel(**inputs):
    inp = {k: np.asarray(v) for k, v in inputs.items()}
    X = inp["X"].astype(np.float32, copy=False)  # [T, N, F0]
    edge_row = inp["edge_row"]
    edge_col = inp["edge_col"]
    edge_val = inp["edge_val"].astype(np.float32, copy=False)
    p = inp["p"].astype(np.float32, copy=False)

    # padded, partition-major, feature-transposed X per core:
    # node n = p*RANKS + i;  XT[core t][p, f*RANKS + i] = X[t, n, f]
    X_pad = np.zeros((T, N_PAD, F0), np.float32)
    X_pad[:, :N] = X
    XT_core = np.ascontiguousarray(
        X_pad.reshape(T, P, RANKS, F0).transpose(0, 1, 3, 2)
    ).reshape(T, P, F0 * RANKS)
    XT_bf = XT_core.astype(bf16)

    Ls, offs, f_pad, col_layout, val_layout, orders, chunks = _edge_layout(
        edge_row, edge_col, edge_val
    )

    # ---- launch 1: yraw_t = X_t @ p
    key1 = ("p1", p.tobytes())
    if key1 not in _cache:
        _cache[key1] = _build_p1(p)
    in1 = [{"XT": XT_core[t]} for t in range(T)]
    res1 = _run(_cache[key1], in1)
    yraw = np.stack([res1.results[t]["yraw"].reshape(-1) for t in range(T)])

    # ---- host: top-16 indices (index selection only)
    yraw16 = np.zeros((16, T), np.float32)
    X16 = np.zeros((16, T * F0), np.float32)
    for t in range(T):
        y = yraw[t][:N]
        cand = np.argpartition(y, -32)[-32:]
        order = cand[np.lexsort((cand, -y[cand]))][:16]
        yraw16[:, t] = y[order]
        X16[:, t * F0 : (t + 1) * F0] = X[t][order]

    # ---- launch 2: GRU chain + s_t = X_t @ (W_t @ lin_w)
    if "p2" not in _cache:
        _cache["p2"] = _build_p2()
    f32 = np.float32
    smalls = np.zeros((16, SMALLS_W), f32)

    def put(name, arr):
        a, b = _COLS[name]
        smalls[:, a:b] = arr

    put("X16", X16)
    put("yraw16", yraw16)
    put("WZT", inp["W_Z"].T.astype(f32))
    put("UZT", inp["U_Z"].T.astype(f32))
    put("BZT", inp["B_Z"].T.astype(f32))
    put("WRT", inp["W_R"].T.astype(f32))
    put("URT", inp["U_R"].T.astype(f32))
    put("BRT", inp["B_R"].T.astype(f32))
    put("WHT", inp["W_H"].T.astype(f32))
    put("UHT", inp["U_H"].T.astype(f32))
    put("BHT", inp["B_H"].T.astype(f32))
    put("Winit", inp["W_init"].astype(f32))
    put("I16", np.eye(16, dtype=f32))
    put("linw_rep", np.tile(inp["lin_w"].astype(f32)[None, :], (16, 1)))
    put("prep16", np.tile(p[None, :], (16, 1)))
    in2 = []
    for t in range(T):
        sm_t = smalls.copy()
        sel = np.zeros((16, T), f32)
        sel[:, t] = 1.0
        a, b = _COLS["sel"]
        sm_t[:, a:b] = sel
        in2.append({"XT": XT_bf[t], "smalls": sm_t})
    res2 = _run(_cache["p2"], in2)
    s_all = np.stack([np.asarray(res2.results[t]["s"]).reshape(-1) for t in range(T)])

    # ---- host re-staging: gather s into the edge layout (index move only),
    # flattened chunk-major so every L3 DMA chunk is one contiguous block
    def _chunk_flat(arr2d):
        return np.concatenate(
            [arr2d[:, c0 : c0 + sum(L * n for (L, n, _) in runs)].reshape(-1)
             for c0, runs in chunks])

    val_bf = val_layout.astype(bf16)
    sg = np.empty((T, P, f_pad), bf16)
    for t in range(T):
        sg[t] = s_all[t][col_layout[t]]
    sgf = [_chunk_flat(sg[t]) for t in range(T)]
    valf = [_chunk_flat(val_bf[t]) for t in range(T)]

    # ---- launch 3: w = val*sg, segmented reduce per rank, + lin_b
    key3 = ("p3", f_pad, tuple(Ls.tolist()))
    if key3 not in _cache:
        _cache[key3] = _build_p3(Ls, chunks, f_pad)
    b_rep = np.full((P, 1), np.float32(inp["lin_b"][0]), np.float32)
    in3 = [{"sg": sgf[t], "val": valf[t], "linb": b_rep} for t in range(T)]
    res3 = _run(_cache[key3], in3)

    # ---- host: un-permute ranks back to node ids
    out = np.zeros((T, N), np.float32)
    for t in range(T):
        y3 = res3.results[t]["y"]  # [P, RANKS]; slot s=128r+p -> y3[p, r]
        flat = np.ascontiguousarray(y3.T).reshape(-1)
        full = np.empty(N_PAD, np.float32)
        full[orders[t]] = flat
        out[t] = full[:N]
    return out


# revision 21
# speedup vs baseline: 1.5948x; 1.5948x over previous
"""EvolveGCN-reg Trainium2 kernel (8 NeuronCores, timestep-parallel).

Math identity: out_t[n] = b + sum_{e: row[e]=n} val[e] * s_t[col[e]],
with s_t = X_t @ u_t, u_t = W_t @ lin_w, and W_t the GRU-evolved 16x16
weight driven by Xs_t (the top-16 rows of X_t by y_t = X_t@p/||p||,
scaled by their y values).

Sharding: core t owns timestep t. Host does index-space layout only
(sharding, gathers, candidate selection); every floating-point op of the
model runs on the NeuronCores.

Launch structure (engine assignment driven by measured rates: DVE 243
G elem/s pure-bf16 / 122 G fp32-touching; PE streams rhs at 128
elem/cycle; GpSimd is slow and contends with DVE for SBUF):
  L1: y_t = X_t @ p on the TENSOR engine from bf16 X in block-diagonal
      layout (8 node-blocks x 16 features on partitions; lhsT [128,8] is
      p masked per block). Host takes the top-32 *candidate indices* per
      timestep (bf16 ranking is within top-17 of exact on this data).
  L2: exact fp32 re-ranking of the 32 candidates on device (PE matvec ->
      top-16 via DVE max/match_replace -> value-match one-hot), Xs built
      by PE from host-staged candidate rows, GRU chain, u_t select, then
      s_t = X_t @ u_t on PE (lhsT = mask * broadcast u). s written bf16.
  L3: w = val*sg (DVE bf16 mult), fold halves (bf16 add at 2x rate),
      segmented reduce per rank, + b. Streams bf16, segment lengths
      padded to even so the fold halves the reduce's input.
"""

import numpy as np
from contextlib import ExitStack

import ml_dtypes

import concourse.bass as bass
import concourse.bacc as bacc
import concourse.tile as tile
from concourse import mybir
from concourse.bass_utils import run_bass_kernel_spmd

dt = mybir.dt
bf16 = ml_dtypes.bfloat16

T, N, E, F0, F1 = 8, 100000, 3200000, 16, 16
NCORES = 8
P = 128
RANKS = (N + P - 1) // P  # 782 (edge-layout ranks for L3)
N_PAD = P * RANKS  # 100096
NBLK = 8  # node blocks in the PE matvec layout
BLK_N = N_PAD // NBLK  # 12512 nodes per block
MMF = 512  # matmul free-dim per instruction
BLK_J = ((BLK_N + MMF - 1) // MMF) * MMF  # 12800 padded block width
NMM = BLK_J // MMF  # 25 matmuls
NCAND = 32
CORE_IDS = list(range(NCORES))

_cache = {}


def _axon_reset():
    try:
        import ctypes

        lib = ctypes.CDLL("/opt/axon/libaxon_pjrt.so")
        lib.axon_reset.restype = ctypes.c_int64
        lib.axon_reset()
    except Exception:
        pass


def _run(nc, in_maps):
    try:
        return run_bass_kernel_spmd(nc, in_maps, core_ids=CORE_IDS)
    except Exception:
        _axon_reset()
        return run_bass_kernel_spmd(nc, in_maps, core_ids=CORE_IDS)


def _emit_pe_matvec(nc, psmv, xp, M, out_sb, dual_queue, xp_ap):
    """out_sb[g, j] = sum_f M[16g+f, g] * X_pe[16g+f, j] via NMM matmuls.

    xp is the [P, BLK_J] bf16 SBUF tile; DMA'd here in 5 chunks (both
    HWDGE queues when this launch is DMA-bound, sync only otherwise).
    """
    CH = 5
    CW = BLK_J // CH  # 2560
    for c in range(CH):
        eng = nc.sync if (dual_queue and c % 2 == 0) or not dual_queue else nc.scalar
        eng.dma_start(xp[:, c * CW : (c + 1) * CW], xp_ap[:, c * CW : (c + 1) * CW])
    for j in range(NMM):
        mv = psmv.tile([NBLK, MMF], dt.float32, tag="mv", name=f"mv{j}")
        nc.tensor.matmul(mv[:], M[:], xp[:, j * MMF : (j + 1) * MMF],
                         start=True, stop=True)
        nc.scalar.copy(out_sb[:, j * MMF : (j + 1) * MMF], mv[:])


# ---------------------------------------------------------------- launch 1
def _build_p1():
    nc = bacc.Bacc("TRN2", target_bir_lowering=False, debug=False)
    xp_ap = nc.dram_tensor("XP", [P, BLK_J], dt.bfloat16, kind="ExternalInput").ap()
    mp_ap = nc.dram_tensor("MP", [P, NBLK], dt.bfloat16, kind="ExternalInput").ap()
    y_ap = nc.dram_tensor("yraw", [NBLK, BLK_J], dt.bfloat16, kind="ExternalOutput").ap()

    with tile.TileContext(nc) as tc, ExitStack() as ctx:
        io = ctx.enter_context(tc.tile_pool(name="io", bufs=1))
        psmv = ctx.enter_context(tc.tile_pool(name="psmv", bufs=4, space="PSUM"))
        mp = io.tile([P, NBLK], dt.bfloat16)
        nc.scalar.dma_start(mp[:], mp_ap[:])
        xp = io.tile([P, BLK_J], dt.bfloat16, tag="xp", name="xp")
        y_sb = io.tile([NBLK, BLK_J], dt.bfloat16, tag="ysb", name="ysb")
        _emit_pe_matvec(nc, psmv, xp, mp, y_sb, True, xp_ap)
        nc.sync.dma_start(y_ap[:], y_sb[:])
    nc.compile()
    return nc


# ---------------------------------------------------------------- launch 2
# packed small-input layout: [32, SMALLS_W] fp32; 16-row blocks live in
# rows 0:16, candidate blocks Xc{tau} use all 32 rows.
_COLS = {}
_off = 0
for _n, _w in ([("WZT", 16), ("UZT", 16), ("BZT", 16), ("WRT", 16),
                ("URT", 16), ("BRT", 16), ("WHT", 16), ("UHT", 16),
                ("BHT", 16), ("Winit", 16), ("I16", 16), ("linw_rep", 16),
                ("sel", 8), ("prep16", 16), ("pcol", 1), ("I16T128", 128)]
               + [(f"Xc{t}", 16) for t in range(T)]
               + [(f"XcT{t}", NCAND) for t in range(T)]):
    _COLS[_n] = (_off, _off + _w)
    _off += _w
SMALLS_W = _off


def _build_p2():
    nc = bacc.Bacc("TRN2", target_bir_lowering=False, debug=False)
    xp_ap = nc.dram_tensor("XP", [P, BLK_J], dt.bfloat16, kind="ExternalInput").ap()
    sm_ap = nc.dram_tensor("smalls", [32, SMALLS_W], dt.float32, kind="ExternalInput").ap()
    msk_ap = nc.dram_tensor("mask", [P, NBLK], dt.float32, kind="ExternalInput").ap()
    s_ap = nc.dram_tensor("s", [NBLK, BLK_J], dt.bfloat16, kind="ExternalOutput").ap()

    with tile.TileContext(nc) as tc, ExitStack() as ctx:
        small = ctx.enter_context(tc.tile_pool(name="small", bufs=1))
        rp = ctx.enter_context(tc.tile_pool(name="rp", bufs=2))
        gru = ctx.enter_context(tc.tile_pool(name="gru", bufs=2))
        ps = ctx.enter_context(tc.tile_pool(name="ps", bufs=2, space="PSUM"))
        psr = ctx.enter_context(tc.tile_pool(name="psr", bufs=2, space="PSUM"))
        psmv = ctx.enter_context(tc.tile_pool(name="psmv", bufs=4, space="PSUM"))
        io = ctx.enter_context(tc.tile_pool(name="io", bufs=1))

        sm = small.tile([32, SMALLS_W], dt.float32)
        nc.scalar.dma_start(sm[:], sm_ap[:])
        msk = small.tile([P, NBLK], dt.float32)
        nc.scalar.dma_start(msk[:], msk_ap[:])
        # X stream on the sync queue only - the scalar queue's sequencer
        # also runs ACT ops, and this launch is not DMA-bound.
        xp = io.tile([P, BLK_J], dt.bfloat16, tag="xp", name="xp")
        for c in range(5):
            CW = BLK_J // 5
            nc.sync.dma_start(xp[:, c * CW : (c + 1) * CW],
                              xp_ap[:, c * CW : (c + 1) * CW])

        # prefetch ACT function tables while the DMAs are in flight
        warm = small.tile([1, 2], dt.float32)
        nc.vector.memset(warm[:], 0.0)
        nc.scalar.activation(warm[:, 0:1], warm[:, 0:1],
                             mybir.ActivationFunctionType.Sigmoid)
        nc.scalar.activation(warm[:, 1:2], warm[:, 1:2],
                             mybir.ActivationFunctionType.Tanh)

        def gi(name):
            a, b = _COLS[name]
            return sm[0:16, a:b]

        def gi32(name):
            a, b = _COLS[name]
            return sm[:, a:b]

        ones1x32 = small.tile([1, NCAND], dt.float32)
        nc.vector.memset(ones1x32[:], 1.0)
        ones11 = small.tile([1, 1], dt.float32)
        nc.vector.memset(ones11[:], 1.0)

        # invp = 1/||p|| on partition 0; replicated to 32 partitions via PE
        psq = small.tile([1, F0], dt.float32)
        nc.scalar.square(psq[:], gi("prep16")[0:1, :])
        pss = small.tile([1, 1], dt.float32)
        nc.vector.tensor_reduce(out=pss[:], in_=psq[:], axis=mybir.AxisListType.X,
                                op=mybir.AluOpType.add)
        pnorm = small.tile([1, 1], dt.float32)
        nc.scalar.sqrt(pnorm[:], pss[:])
        invp = small.tile([1, 1], dt.float32)
        nc.vector.reciprocal(invp[:], pnorm[:])
        scr0 = psr.tile([P, 512], dt.float32, tag="scr", name="scr_misc")
        invp32_ps = scr0[0:NCAND, 0:1]
        nc.tensor.matmul(invp32_ps, ones1x32[:], invp[:], start=True, stop=True)
        invp32 = small.tile([NCAND, 1], dt.float32)
        nc.scalar.copy(invp32[:], invp32_ps)

        # exact fp32 re-rank of the NCAND candidates per tau -> Xs_tau.
        # Emitted interleaved with the GRU steps (refine tau+1 between GRU
        # tau and tau+1) so the in-order ACT queue doesn't stall tau 0.
        Xs = [None] * T

        def emit_refine(tau):
            # all small PSUM results share one bank-sized scratch tile
            scr = psr.tile([P, 512], dt.float32, tag="scr", name=f"scr{tau}")
            yct_ps = scr[0:1, 0:NCAND]
            nc.tensor.matmul(yct_ps, gi("pcol"), gi(f"XcT{tau}"),
                             start=True, stop=True)
            yct = rp.tile([1, NCAND], dt.float32, tag="yct_sb", name=f"yct{tau}")
            nc.scalar.copy(yct[:], yct_ps)
            m1 = rp.tile([1, 8], dt.float32, tag="m1", name=f"m1{tau}")
            nc.vector.max(m1[:], yct[:])
            y2 = rp.tile([1, NCAND], dt.float32, tag="y2", name=f"y2{tau}")
            nc.vector.match_replace(out=y2[:], in_to_replace=m1[:],
                                    in_values=yct[:], imm_value=-3e38)
            m2 = rp.tile([1, 8], dt.float32, tag="m2", name=f"m2{tau}")
            nc.vector.max(m2[:], y2[:])
            yk = rp.tile([1, F1], dt.float32, tag="yk", name=f"yk{tau}")
            nc.scalar.copy(yk[:, 0:8], m1[:])
            nc.scalar.copy(yk[:, 8:16], m2[:])
            # yc column [NCAND,1] (PE transpose) and yk broadcast [NCAND,16]
            yc_ps = scr[0:NCAND, 64:65]
            nc.tensor.matmul(yc_ps, yct[:], ones11[:], start=True, stop=True)
            yc = rp.tile([NCAND, 1], dt.float32, tag="yc_sb", name=f"yc{tau}")
            nc.scalar.copy(yc[:], yc_ps)
            ykb_ps = scr[0:NCAND, 96:112]
            nc.tensor.matmul(ykb_ps, ones1x32[:], yk[:], start=True, stop=True)
            ykb = rp.tile([NCAND, F1], dt.float32, tag="ykb_sb", name=f"ykb{tau}")
            nc.scalar.copy(ykb[:], ykb_ps)
            # Sy[c,k] = 1{yc[c]==yk[k]} * yk[k] * invp  (exact fp32 match)
            S = rp.tile([NCAND, F1], dt.float32, tag="S", name=f"S{tau}")
            nc.vector.tensor_scalar(S[:], ykb[:], yc[:], None,
                                    mybir.AluOpType.is_equal)
            Sy = rp.tile([NCAND, F1], dt.float32, tag="Sy", name=f"Sy{tau}")
            nc.vector.scalar_tensor_tensor(
                out=Sy[:], in0=S[:], scalar=invp32[:], in1=ykb[:],
                op0=mybir.AluOpType.mult, op1=mybir.AluOpType.mult)
            xs_ps = scr[0:F0, 128:144]
            nc.tensor.matmul(xs_ps, gi32(f"Xc{tau}"), Sy[:], start=True, stop=True)
            xs = gru.tile([F0, F1], dt.float32, tag="xs_sb", name=f"xs{tau}")
            nc.scalar.copy(xs[:], xs_ps)
            Xs[tau] = xs

        # GRU chain; bias folded into the PE accumulation group
        u_cols = small.tile([16, T], dt.float32)
        W = gi("Winit")
        emit_refine(0)
        for tau in range(T):
            def gate(wt, ut, bt, rhs2, func, tag):
                acc = ps.tile([16, 16], dt.float32, tag="mm", name=f"mm{tag}{tau}")
                nc.tensor.matmul(acc[:], gi(wt), Xs[tau][:], start=True, stop=False)
                nc.tensor.matmul(acc[:], gi(bt), gi("I16"), start=False, stop=False)
                nc.tensor.matmul(acc[:], gi(ut), rhs2[:], start=False, stop=True)
                g = gru.tile([16, 16], dt.float32, tag=f"g{tag}", name=f"g{tag}{tau}")
                nc.scalar.activation(g[:], acc[:], func)
                return g

            Zg = gate("WZT", "UZT", "BZT", W, mybir.ActivationFunctionType.Sigmoid, "z")
            Rg = gate("WRT", "URT", "BRT", W, mybir.ActivationFunctionType.Sigmoid, "r")
            RW = gru.tile([16, 16], dt.float32, tag="rw", name=f"rw{tau}")
            nc.vector.tensor_tensor(out=RW[:], in0=Rg[:], in1=W[:],
                                    op=mybir.AluOpType.mult)
            Ht = gate("WHT", "UHT", "BHT", RW, mybir.ActivationFunctionType.Tanh, "h")

            HmW = gru.tile([16, 16], dt.float32, tag="hmw", name=f"hmw{tau}")
            nc.vector.tensor_tensor(out=HmW[:], in0=Ht[:], in1=W[:],
                                    op=mybir.AluOpType.subtract)
            ZH = gru.tile([16, 16], dt.float32, tag="zh", name=f"zh{tau}")
            nc.vector.tensor_tensor(out=ZH[:], in0=Zg[:], in1=HmW[:],
                                    op=mybir.AluOpType.mult)
            Wn = gru.tile([16, 16], dt.float32, tag=f"w{tau}", name=f"w{tau}")
            nc.vector.tensor_tensor(out=Wn[:], in0=W[:], in1=ZH[:],
                                    op=mybir.AluOpType.add)
            W = Wn

            um = gru.tile([16, 16], dt.float32, tag="um", name=f"um{tau}")
            nc.vector.tensor_tensor(out=um[:], in0=W[:], in1=gi("linw_rep"),
                                    op=mybir.AluOpType.mult)
            nc.vector.tensor_reduce(out=u_cols[:, tau : tau + 1], in_=um[:],
                                    axis=mybir.AxisListType.X, op=mybir.AluOpType.add)
            if tau + 1 < T:
                emit_refine(tau + 1)

        # select this core's u via one-hot mask; expand to the block-diag M
        usm = small.tile([16, T], dt.float32)
        nc.vector.tensor_tensor(out=usm[:], in0=u_cols[:], in1=gi("sel"),
                                op=mybir.AluOpType.mult)
        u_sel = small.tile([16, 1], dt.float32)
        nc.vector.tensor_reduce(out=u_sel[:], in_=usm[:], axis=mybir.AxisListType.X,
                                op=mybir.AluOpType.add)
        scru = psr.tile([P, 512], dt.float32, tag="scr", name="scr_u")
        u128_ps = scru[:, 0:1]
        nc.tensor.matmul(u128_ps, gi("I16T128"), u_sel[:], start=True, stop=True)
        u128 = small.tile([P, 1], dt.float32)
        nc.scalar.copy(u128[:], u128_ps)
        M = small.tile([P, NBLK], dt.bfloat16)
        nc.vector.tensor_scalar_mul(M[:], msk[:], u128[:])

        s_sb = io.tile([NBLK, BLK_J], dt.bfloat16, tag="ssb", name="ssb")
        for j in range(NMM):
            mv = psmv.tile([NBLK, MMF], dt.float32, tag="mv", name=f"mv{j}")
            nc.tensor.matmul(mv[:], M[:], xp[:, j * MMF : (j + 1) * MMF],
                             start=True, stop=True)
            nc.scalar.copy(s_sb[:, j * MMF : (j + 1) * MMF], mv[:])
        nc.sync.dma_start(s_ap[:], s_sb[:])
    nc.compile()
    return nc


# ---------------------------------------------------------------- launch 3
def _build_p3(Ls, chunks, f_pad):
    nc = bacc.Bacc("TRN2", target_bir_lowering=False, debug=False)
    in_dt = dt.bfloat16
    tot = sum(sum(L * cnt for (L, cnt, _) in runs) for _, runs in chunks) * P
    sg_ap = nc.dram_tensor("sg", [tot], in_dt, kind="ExternalInput").ap()
    val_ap = nc.dram_tensor("val", [tot], in_dt, kind="ExternalInput").ap()
    b_ap = nc.dram_tensor("linb", [P, 1], dt.float32, kind="ExternalInput").ap()
    y_ap = nc.dram_tensor("y", [P, RANKS], dt.float32, kind="ExternalOutput").ap()

    with tile.TileContext(nc) as tc, ExitStack() as ctx:
        io = ctx.enter_context(tc.tile_pool(name="io", bufs=3))
        yp = ctx.enter_context(tc.tile_pool(name="y", bufs=1))
        b_t = yp.tile([P, 1], dt.float32)
        nc.scalar.dma_start(b_t[:], b_ap[:])
        y_t = yp.tile([P, RANKS], dt.float32)
        yb = yp.tile([P, RANKS], dt.float32)
        for ci, (col0, runs) in enumerate(chunks):
            ncols = sum(L * cnt for (L, cnt, _) in runs)
            sg_t = io.tile([P, ncols], in_dt, tag="sg", name="sg_t")
            nc.sync.dma_start(
                sg_t[:], sg_ap[col0 * P : (col0 + ncols) * P].rearrange(
                    "(p j) -> p j", j=ncols))
            val_t = io.tile([P, ncols], in_dt, tag="val", name="val_t")
            nc.scalar.dma_start(
                val_t[:], val_ap[col0 * P : (col0 + ncols) * P].rearrange(
                    "(p j) -> p j", j=ncols))
            w_t = io.tile([P, ncols], in_dt, tag="w", name="w_t")
            nc.vector.tensor_tensor(out=w_t[:], in0=sg_t[:], in1=val_t[:],
                                    op=mybir.AluOpType.mult)
            # bf16 fold (2x rate) halves each segment, then reduce
            wf = io.tile([P, ncols // 2], in_dt, tag="wf", name="wf_t")
            c = 0
            cf = 0
            for L, cnt, rank0 in runs:
                h = L // 2
                seg3 = w_t[:, c : c + cnt * L].rearrange("p (r l) -> p r l", l=L)
                dst = wf[:, cf : cf + cnt * h].rearrange("p (r l) -> p r l", l=h)
                nc.vector.tensor_tensor(out=dst, in0=seg3[:, :, 0:h],
                                        in1=seg3[:, :, h:L], op=mybir.AluOpType.add)
                nc.vector.tensor_reduce(
                    out=y_t[:, rank0 : rank0 + cnt], in_=dst,
                    axis=mybir.AxisListType.X, op=mybir.AluOpType.add,
                )
                c += cnt * L
                cf += cnt * h
            r0 = runs[0][2]
            r1 = runs[-1][2] + runs[-1][1]
            nc.vector.tensor_scalar_add(yb[:, r0:r1], y_t[:, r0:r1], b_t[:])
            eng = nc.sync if ci % 2 == 0 else nc.scalar
            eng.dma_start(y_ap[:, r0:r1], yb[:, r0:r1])
    nc.compile()
    return nc


# ------------------------------------------------------------ host layout
def _edge_layout(edge_row, edge_col, edge_val):
    """Degree-sorted, rank-equalized destination layout shared across T.
    Segment lengths padded to even so L3 can fold-halve before reducing."""
    degs = np.zeros((T, N_PAD), np.int64)
    orders = np.zeros((T, N_PAD), np.int64)
    for t in range(T):
        deg = np.bincount(edge_row[t].astype(np.int64), minlength=N_PAD)
        degs[t] = deg
        orders[t] = np.argsort(-deg, kind="stable")
    rank_max = np.zeros((T, RANKS), np.int64)
    for t in range(T):
        rank_max[t] = degs[t][orders[t]].reshape(RANKS, P).max(1)
    Ls = rank_max.max(0)
    Ls = (Ls + 1) // 2 * 2  # even for the fold
    Ls = np.maximum.accumulate(Ls[::-1])[::-1]  # enforce non-increasing
    Ls = np.maximum(Ls, 2)
    offs = np.zeros(RANKS + 1, np.int64)
    offs[1:] = np.cumsum(Ls)
    f_pad = int(-(-offs[-1] // 8) * 8)

    col_layout = np.zeros((T, P, f_pad), np.int32)
    val_layout = np.zeros((T, P, f_pad), np.float32)
    for t in range(T):
        row = edge_row[t].astype(np.int64)
        order = orders[t]
        slot_of_node = np.empty(N_PAD, np.int64)
        slot_of_node[order] = np.arange(N_PAD)
        ord_e = np.argsort(row, kind="stable")
        rows_s = row[ord_e]
        deg = degs[t]
        node_start = np.zeros(N_PAD, np.int64)
        node_start[1:] = np.cumsum(deg)[:-1]
        k = np.arange(E, dtype=np.int64) - node_start[rows_s]
        s = slot_of_node[rows_s]
        p_idx = s % P
        r_idx = s // P
        pos = offs[r_idx] + k
        col_layout[t, p_idx, pos] = edge_col[t][ord_e]
        val_layout[t, p_idx, pos] = edge_val[t][ord_e]

    # chunk schedule shared across cores (~3200 bf16 cols per chunk)
    FC = 3200
    chunks = []
    cur, cur_cols, col0, r = [], 0, 0, 0
    while r < RANKS:
        L = int(Ls[r])
        cnt = 0
        while r + cnt < RANKS and Ls[r + cnt] == L and cur_cols + (cnt + 1) * L <= FC:
            cnt += 1
        if cnt == 0:
            chunks.append((col0, cur))
            col0 += cur_cols
            cur, cur_cols = [], 0
            continue
        cur.append((L, cnt, r))
        cur_cols += cnt * L
        r += cnt
    if cur:
        chunks.append((col0, cur))
    return Ls, offs, f_pad, col_layout, val_layout, orders, chunks


# ------------------------------------------------------------------ kernel
def kernel(**inputs):
    inp = {k: np.asarray(v) for k, v in inputs.items()}
    X = inp["X"].astype(np.float32, copy=False)  # [T, N, F0]
    edge_row = inp["edge_row"]
    edge_col = inp["edge_col"]
    edge_val = inp["edge_val"].astype(np.float32, copy=False)
    p = inp["p"].astype(np.float32, copy=False)

    # block-diagonal PE layout: node n = g*BLK_N + j lives in column j of
    # block g; X_pe[16g+f, j] = X[t, n, f]
    X_pad = np.zeros((T, N_PAD, F0), np.float32)
    X_pad[:, :N] = X
    XP = np.zeros((T, P, BLK_J), bf16)
    XP[:, :, :BLK_N] = np.ascontiguousarray(
        X_pad.reshape(T, NBLK, BLK_N, F0).transpose(0, 1, 3, 2)
    ).reshape(T, P, BLK_N).astype(bf16)

    mask = np.zeros((P, NBLK), np.float32)
    for g in range(NBLK):
        mask[16 * g : 16 * (g + 1), g] = 1.0
    MP = (mask * np.tile(p, NBLK)[:, None]).astype(bf16)

    Ls, offs, f_pad, col_layout, val_layout, orders, chunks = _edge_layout(
        edge_row, edge_col, edge_val
    )

    # ---- launch 1: y_t = X_t @ p (bf16; candidate ranking only)
    if "p1" not in _cache:
        _cache["p1"] = _build_p1()
    in1 = [{"XP": XP[t], "MP": MP} for t in range(T)]
    res1 = _run(_cache["p1"], in1)

    # ---- host: top-NCAND candidate indices per tau (index move only)
    f32 = np.float32
    smalls = np.zeros((32, SMALLS_W), f32)

    def put(name, arr, rows=16):
        a, b = _COLS[name]
        smalls[0:rows, a:b] = arr

    for t in range(T):
        y = np.asarray(res1.results[t]["yraw"])[:, :BLK_N].reshape(-1)[:N]
        y = y.astype(f32)
        cand = np.argpartition(y, -NCAND)[-NCAND:]
        Xc = X[t][cand]  # [NCAND, F0]
        put(f"Xc{t}", Xc, rows=NCAND)
        put(f"XcT{t}", Xc.T)

    # ---- launch 2: candidate re-rank + GRU + s_t = X_t @ u_t
    if "p2" not in _cache:
        _cache["p2"] = _build_p2()
    put("WZT", inp["W_Z"].T.astype(f32))
    put("UZT", inp["U_Z"].T.astype(f32))
    put("BZT", inp["B_Z"].T.astype(f32))
    put("WRT", inp["W_R"].T.astype(f32))
    put("URT", inp["U_R"].T.astype(f32))
    put("BRT", inp["B_R"].T.astype(f32))
    put("WHT", inp["W_H"].T.astype(f32))
    put("UHT", inp["U_H"].T.astype(f32))
    put("BHT", inp["B_H"].T.astype(f32))
    put("Winit", inp["W_init"].astype(f32))
    put("I16", np.eye(16, dtype=f32))
    put("linw_rep", np.tile(inp["lin_w"].astype(f32)[None, :], (16, 1)))
    put("prep16", np.tile(p[None, :], (16, 1)))
    put("pcol", p[:, None])
    put("I16T128", np.tile(np.eye(16, dtype=f32), (1, NBLK)))
    in2 = []
    for t in range(T):
        sm_t = smalls.copy()
        sel = np.zeros((16, T), f32)
        sel[:, t] = 1.0
        a, b = _COLS["sel"]
        sm_t[0:16, a:b] = sel
        in2.append({"XP": XP[t], "smalls": sm_t, "mask": mask})
    res2 = _run(_cache["p2"], in2)
    s_all = np.stack([
        np.asarray(res2.results[t]["s"])[:, :BLK_N].reshape(-1) for t in range(T)
    ])  # [T, N_PAD] bf16, node-indexed

    # ---- host re-staging: gather s into the edge layout (index move only)
    def _chunk_flat(arr2d):
        return np.concatenate(
            [arr2d[:, c0 : c0 + sum(L * n for (L, n, _) in runs)].reshape(-1)
             for c0, runs in chunks])

    val_bf = val_layout.astype(bf16)
    sg = np.empty((T, P, f_pad), bf16)
    for t in range(T):
        sg[t] = s_all[t][col_layout[t]]
    sgf = [_chunk_flat(sg[t]) for t in range(T)]
    valf = [_chunk_flat(val_bf[t]) for t in range(T)]

    # ---- launch 3: w = val*sg, fold, segmented reduce per rank, + lin_b
    key3 = ("p3", f_pad, tuple(Ls.tolist()))
    if key3 not in _cache:
        _cache[key3] = _build_p3(Ls, chunks, f_pad)
    b_rep = np.full((P, 1), np.float32(inp["lin_b"][0]), np.float32)
    in3 = [{"sg": sgf[t], "val": valf[t], "linb": b_rep} for t in range(T)]
    res3 = _run(_cache[key3], in3)

    # ---- host: un-permute ranks back to node ids
    out = np.zeros((T, N), np.float32)
    for t in range(T):
        y3 = res3.results[t]["y"]  # [P, RANKS]; slot s=P*r+p -> y3[p, r]
        flat = np.ascontiguousarray(y3.T).reshape(-1)
        full = np.empty(N_PAD, np.float32)
        full[orders[t]] = flat
        out[t] = full[:N]
    return out
